# revision 1
# baseline (speedup 1.0000x reference)
"""Multi-head self-attention (B=2, S=2048, E=1024, H=16, D=64) on 8 NeuronCores.

Sharding: core c -> (batch b = c // 4, head group g = c % 4).  Each core
computes Q/K/V projections for its 4 heads (column-parallel), attention, and
a partial output projection (row-parallel); the host sums the 4 partials per
batch.  All device activations live in "transposed space" (feature on the
partition dim) so every matmul contracts along partitions with no on-device
transposes:

  Q^T = Wq_g^T @ X^T          [256, 2048]  (e-chunk accumulated, + bq)
  K^T = Wk_g^T @ X^T          [256, 2048]
  V   = X @ Wv_g              [2048, 256]  (natural; ones column appended)
  S^T = K_h @ Q_h^T / 8       [2048, 2048] per head (computed tile-wise)
  P^T = exp(S^T)              (softmax without max-subtraction: scores ~N(0,1))
  O'^T = [V_h | 1]^T @ P^T    [65, q]  (row 64 = softmax denominators)
  O^T  = O'[0:64] / O'[64]    (DVE reciprocal + GpSimd partition broadcast)
  Y^T  = Wo_g^T @ O^T         [1024, 2048] partial, host-summed per batch

bv and bo are folded on the host (exact: softmax rows sum to 1, so
attn(V + bv) = attn(V) + bv, and the output projection is linear).
"""

from contextlib import ExitStack

import numpy as np

import concourse.bass as bass
import concourse.tile as tile
from concourse import bacc, mybir
from concourse.bass_utils import run_bass_kernel_spmd

B, S, E, H, D = 2, 2048, 1024, 16, 64
NCORES = 8
GH = 4            # heads per core
DC = GH * D       # head-dim columns per core (256)
EC = E // 128     # 8 e-chunks
KC = S // 128     # 16 k-chunks
F32 = mybir.dt.float32
MM_DT = mybir.dt.float16    # full-speed 16-bit matmul path (10-bit mantissa)
EXP_FUNC = mybir.ActivationFunctionType.Exp
SCALE = 1.0 / np.sqrt(np.float32(D))


def _mm(ap):
    return ap


def round_f32r(a):
    # Host-side conversion to the matmul dtype (RNE)
    if MM_DT == mybir.dt.float16:
        return np.ascontiguousarray(a, np.float32).astype(np.float16)
    if MM_DT == mybir.dt.bfloat16:
        import ml_dtypes
        return np.ascontiguousarray(a, np.float32).astype(ml_dtypes.bfloat16)
    if MM_DT == mybir.dt.float32r:
        u = np.ascontiguousarray(a, np.float32).view(np.uint32)
        u = ((u.astype(np.uint64) + 0x800) & 0xFFFFF000).astype(np.uint32)
        return u.view(np.float32)
    return np.ascontiguousarray(a, np.float32)


DEBUG_DUMPS = False


def _emit(nc, tc, ctx, xT, wq, wk, wv, wo, bq, bk, yT, dbg=None):
    sb_big = ctx.enter_context(tc.tile_pool(name="sb_big", bufs=1))
    sb_p = ctx.enter_context(tc.tile_pool(name="sb_p", bufs=28))
    sb_norm = ctx.enter_context(tc.tile_pool(name="sb_norm", bufs=4))
    sb_y = ctx.enter_context(tc.tile_pool(name="sb_y", bufs=2))
    ps_big = ctx.enter_context(tc.tile_pool(name="ps_big", bufs=2, space="PSUM"))
    ps_acc = ctx.enter_context(tc.tile_pool(name="ps_acc", bufs=2, space="PSUM"))

    xT_t = sb_big.tile([128, EC, S], MM_DT)
    wq_t = sb_big.tile([128, EC, DC], MM_DT)
    wk_t = sb_big.tile([128, EC, DC], MM_DT)
    wv_t = sb_big.tile([128, EC, DC], MM_DT)
    wo_t = sb_big.tile([128, 2, E], MM_DT)
    bqk_t = sb_big.tile([1, 2, DC], MM_DT)
    ones_t = sb_big.tile([1, 512], MM_DT)
    qT_t = sb_big.tile([128, 2, S], MM_DT)
    kT_t = sb_big.tile([128, 2, S], MM_DT)
    v_t = sb_big.tile([128, KC, GH, D + 1], MM_DT)
    o_t = sb_big.tile([128, 2, S], MM_DT)

    # Inputs are host-permuted to the exact SBUF layouts, so every load is a
    # dense per-partition-contiguous copy (cheap descriptors); issues are
    # spread across engine queues to parallelize DMA setup.
    nc.scalar.dma_start(out=wq_t[:, :, :],
                        in_=wq.rearrange("p (c d) -> p c d", c=EC))
    nc.scalar.dma_start(out=wk_t[:, :, :],
                        in_=wk.rearrange("p (c d) -> p c d", c=EC))
    for ec in range(EC):
        eng = nc.sync if ec % 2 == 0 else nc.gpsimd
        eng.dma_start(out=xT_t[:, ec, :], in_=xT[:, ec * S:(ec + 1) * S])
    nc.sync.dma_start(out=wv_t[:, :, :],
                        in_=wv.rearrange("p (c d) -> p c d", c=EC))
    nc.gpsimd.dma_start(out=wo_t[:, :, :],
                        in_=wo.rearrange("p (c e) -> p c e", c=2))
    nc.gpsimd.dma_start(out=bqk_t[:, 0, :], in_=bq[None, :])
    nc.gpsimd.dma_start(out=bqk_t[:, 1, :], in_=bk[None, :])
    nc.vector.memset(ones_t[:, :], 1.0)
    for kc in range(KC):
        nc.vector.memset(v_t[:, kc, :, D:D + 1], 1.0)


    def qk_part(dc, proj, sc, half, state={}):
        # psum[d, s] += W[e, d].T @ X^T[e, s]   (+ bias via K=1 matmul),
        # emitted in two halves so filler bursts stay small
        w_t, dst = ((wq_t, qT_t), (wk_t, kT_t))[proj]
        if half == 0:
            state[(dc, proj, sc)] = ps_big.tile(
                [128, 512], F32, tag="big", name="ps_qk")
        ps = state[(dc, proj, sc)]
        ecs = range(EC // 2) if half == 0 else range(EC // 2, EC)
        for ec in ecs:
            nc.tensor.matmul(
                ps[:, :],
                lhsT=w_t[:, ec, dc * 128:(dc + 1) * 128],
                rhs=xT_t[:, ec, sc * 512:(sc + 1) * 512],
                start=(ec == 0), stop=False)
        if half == 1:
            nc.tensor.matmul(
                ps[:, :],
                lhsT=bqk_t[:, proj, dc * 128:(dc + 1) * 128],
                rhs=ones_t[:, :],
                start=False, stop=True)
            nc.vector.tensor_copy(
                out=dst[:, dc, sc * 512:(sc + 1) * 512], in_=ps[:, :])
            del state[(dc, proj, sc)]

    def qk_group(dc, proj, sc):
        qk_part(dc, proj, sc, 0)
        qk_part(dc, proj, sc, 1)

    def v_proj():
        # psum[s, d] += X^T[e, s].T @ Wv[e, d]
        for kc in range(KC):
            ps = ps_acc.tile([128, 512], F32, tag="acc", bufs=4, name="ps_v")
            for ec in range(EC):
                nc.tensor.matmul(
                    ps[:, 0:DC],
                    lhsT=xT_t[:, ec, kc * 128:(kc + 1) * 128],
                    rhs=wv_t[:, ec, :],
                    start=(ec == 0), stop=(ec == EC - 1))
            nc.vector.tensor_copy(
                out=v_t[:, kc, :, 0:D],
                in_=ps[:, 0:DC].rearrange("p (h d) -> p h d", h=GH))

    def attention_scores(qc, hc, kcs=None):
        # Head pair (2*hc, 2*hc+1): head hp=0 on SBUF partitions 0-63, hp=1
        # on 64-127, so the two scores matmuls run as independent 64x128 PE
        # tiles and one ACTIVATE covers both heads' exp.
        pTs = []
        for kc in (kcs if kcs is not None else range(KC)):
            sco = ps_big.tile([128, 2, 512], F32, tag="big", name="sco")
            for hp in range(2):
                po = hp * 64
                nc.tensor.matmul(
                    sco[:, hp, :],
                    lhsT=kT_t[po:po + 64, hc, kc * 128:(kc + 1) * 128],
                    rhs=qT_t[po:po + 64, hc, qc * 512:(qc + 1) * 512],
                    start=True, stop=True)
            pT = sb_p.tile([128, 2, 512], MM_DT)
            nc.scalar.activation(
                out=pT[:, :, :], in_=sco[:, :, :], func=EXP_FUNC,
                scale=float(SCALE))
            pTs.append(pT)
        return pTs

    def pv_alloc():
        return [ps_acc.tile([128, 512], F32, tag="acc", bufs=4, name=f"acc{j}")
                for j in range(2)]

    def pv_kc(accs, hc, pTs, kc):
        for hp in range(2):
            h = 2 * hc + hp
            nc.tensor.matmul(
                accs[hp][0:D + 1, :],
                lhsT=v_t[:, kc, h, :],
                rhs=pTs[kc][:, hp, :],
                start=(kc == 0), stop=(kc == KC - 1))

    def attention_norm(qc, hc, accs):
        for hp in range(2):
            po = hp * 64
            rs = sb_norm.tile([1, 512], F32, tag="rs")
            nc.vector.tensor_copy(out=rs[:, :], in_=accs[hp][D:D + 1, :])
            inv_r = sb_norm.tile([1, 512], F32, tag="inv")
            nc.vector.reciprocal_approx_fast(out=inv_r[:, :], in_=rs[:, :])
            brd = sb_norm.tile([64, 512], F32, tag="brd")
            nc.gpsimd.partition_broadcast(brd[:, :], inv_r[:, :])
            nc.vector.tensor_mul(
                o_t[po:po + 64, hc, qc * 512:(qc + 1) * 512],
                accs[hp][0:D, :],
                brd[:, :])

    def attention_pv(qc, hc, pTs):
        accs = pv_alloc()
        for kc in range(KC):
            pv_kc(accs, hc, pTs, kc)
        attention_norm(qc, hc, accs)

    def y_group(qc, ec, tag="acc", bufs=4, copy_eng=None):
        # psum[e, s] += Wo[c, e].T @ O^T[c, s] for chunk (ec, qc)
        yp = ps_acc.tile([128, 512], F32, tag=tag, bufs=bufs, name="yp")
        for cc in range(2):
            nc.tensor.matmul(
                yp[:, :],
                lhsT=wo_t[:, cc, ec * 128:(ec + 1) * 128],
                rhs=o_t[:, cc, qc * 512:(qc + 1) * 512],
                start=(cc == 0), stop=(cc == 1))
        ys = sb_y.tile([128, 512], F32)
        if copy_eng == "scalar":
            nc.scalar.copy(out=ys[:, :], in_=yp[:, :])
        else:
            nc.vector.tensor_copy(out=ys[:, :], in_=yp[:, :])
        nc.sync.dma_start(
            out=yT[ec * 128:(ec + 1) * 128, qc * 512:(qc + 1) * 512],
            in_=ys[:, :])

    def y_proj(qc):
        for ec in range(EC):
            y_group(qc, ec)

    # Software-pipelined emission (= Tile priority order).  The exp stream
    # drives the schedule: each attention block emits scores+exp for (qc, hc)
    # at top priority while the PREVIOUS block's PV matmuls and filler work
    # (remaining projections, output-projection chunks) weave in at kc
    # granularity, so ScalarE never starves.
    blocks = [(0, 0), (1, 0), (2, 0), (3, 0), (0, 1), (1, 1), (2, 1), (3, 1)]

    def qk1(proj, sc, half):
        return lambda: qk_part(1, proj, sc, half)

    def qk0(proj, sc, half):
        return lambda: qk_part(0, proj, sc, half)

    # filler generators keyed by block index: list of (after_kc, fn)
    fillers = {
        1: [(3, qk0(0, 2, 0)), (5, qk0(0, 2, 1)),
            (11, qk0(0, 3, 0)), (13, qk0(0, 3, 1))],
        2: [(1, qk1(0, 0, 0)), (3, qk1(0, 0, 1)),
            (5, qk1(1, 0, 0)), (7, qk1(1, 0, 1)),
            (9, qk1(0, 1, 0)), (11, qk1(0, 1, 1)),
            (13, qk1(1, 1, 0)), (15, qk1(1, 1, 1))],
        3: [(1, qk1(0, 2, 0)), (3, qk1(0, 2, 1)),
            (5, qk1(1, 2, 0)), (7, qk1(1, 2, 1)),
            (9, qk1(0, 3, 0)), (11, qk1(0, 3, 1)),
            (13, qk1(1, 3, 0)), (15, qk1(1, 3, 1))],
        6: [(2 * i + 1, (lambda e: lambda: y_group(0, e))(i)) for i in range(EC)],
        7: [(2 * i + 1, (lambda e: lambda: y_group(1, e))(i)) for i in range(EC)],
    }

    # staged startup: emit first-block scores as soon as each kT s-chunk's
    # projection is emitted, so the exp stream starts ~20us earlier
    qk_group(0, 0, 0)
    qk_group(0, 1, 0)
    pts_prev = attention_scores(0, 0, range(0, 4))
    qk_group(0, 1, 1)
    pts_prev += attention_scores(0, 0, range(4, 8))
    qk_group(0, 1, 2)
    pts_prev += attention_scores(0, 0, range(8, 12))
    qk_group(0, 1, 3)
    pts_prev += attention_scores(0, 0, range(12, 16))
    qk_group(0, 0, 1)
    v_proj()
    prev_block = (0, 0)
    for bi in range(1, len(blocks)):
        qc, hc = blocks[bi]
        pqc, phc = prev_block
        accs = pv_alloc()
        pts_cur = []
        fl = dict((k, f) for k, f in fillers.get(bi, []))
        for kc in range(KC):
            pts_cur += attention_scores(qc, hc, [kc])
            pv_kc(accs, phc, pts_prev, kc)
            if kc in fl:
                fl[kc]()
        attention_norm(pqc, phc, accs)
        pts_prev = pts_cur
        prev_block = (qc, hc)
    # final block: PV + norm + remaining output projection
    accs = pv_alloc()
    for kc in range(KC):
        pv_kc(accs, prev_block[1], pts_prev, kc)
        if kc % 2 == 1:
            y_group(2, kc // 2)
    attention_norm(prev_block[0], prev_block[1], accs)
    for ec in range(EC):
        y_group(3, ec, copy_eng="scalar" if ec % 2 else None)

    if dbg is not None:
        for name, t in (("qT", qT_t), ("kT", kT_t), ("o", o_t)):
            f = sb_big.tile([128, 2, S], F32, name=f"dump_{name}")
            nc.vector.tensor_copy(out=f[:, :, :], in_=t[:, :, :])
            nc.sync.dma_start(out=dbg[name], in_=f.rearrange("p a b -> p (a b)"))
        fv = sb_big.tile([128, KC, GH, D + 1], F32, name="dump_v")
        nc.vector.tensor_copy(out=fv[:, :, :, :], in_=v_t[:, :, :, :])
        nc.sync.dma_start(out=dbg["v"], in_=fv.rearrange("p a b c -> p (a b c)"))


_cached_nc = None


def _build():
    nc = bacc.Bacc(trn_type="TRN2", target_bir_lowering=False)
    xT = nc.dram_tensor("xT", [128, EC * S], MM_DT, kind="ExternalInput").ap()
    wq = nc.dram_tensor("wq", [128, EC * DC], MM_DT, kind="ExternalInput").ap()
    wk = nc.dram_tensor("wk", [128, EC * DC], MM_DT, kind="ExternalInput").ap()
    wv = nc.dram_tensor("wv", [128, EC * DC], MM_DT, kind="ExternalInput").ap()
    wo = nc.dram_tensor("wo", [128, 2 * E], MM_DT, kind="ExternalInput").ap()
    bq = nc.dram_tensor("bq", [DC], MM_DT, kind="ExternalInput").ap()
    bk = nc.dram_tensor("bk", [DC], MM_DT, kind="ExternalInput").ap()
    yT = nc.dram_tensor("yT", [E, S], F32, kind="ExternalOutput").ap()
    dbg = None
    if DEBUG_DUMPS:
        dbg = {
            "qT": nc.dram_tensor("dbg_qT", [128, 2 * S], F32, kind="ExternalOutput").ap(),
            "kT": nc.dram_tensor("dbg_kT", [128, 2 * S], F32, kind="ExternalOutput").ap(),
            "o": nc.dram_tensor("dbg_o", [128, 2 * S], F32, kind="ExternalOutput").ap(),
            "v": nc.dram_tensor("dbg_v", [128, KC * GH * (D + 1)], F32, kind="ExternalOutput").ap(),
        }
    with tile.TileContext(nc) as tc:
        with ExitStack() as ctx:
            _emit(nc, tc, ctx, xT, wq, wk, wv, wo, bq, bk, yT, dbg)
    nc.compile()
    return nc


def get_nc():
    global _cached_nc
    if _cached_nc is None:
        _cached_nc = _build()
    return _cached_nc


def make_in_maps(inputs, wq, bq, wk, bk, wv, wo):
    in_maps = []
    for c in range(NCORES):
        b, g = divmod(c, GH)
        sl = slice(g * DC, (g + 1) * DC)
        def perm(a):
            # [C*128, N] -> [128, C*N] with SBUF chunk-major free dim
            cN = a.shape[0] // 128
            return np.ascontiguousarray(
                a.reshape(cN, 128, a.shape[1]).transpose(1, 0, 2).reshape(
                    128, cN * a.shape[1]))

        in_maps.append({
            "xT": round_f32r(perm(np.ascontiguousarray(inputs[b].T))),
            "wq": round_f32r(perm(wq[:, sl])),
            "wk": round_f32r(perm(wk[:, sl])),
            "wv": round_f32r(perm(wv[:, sl])),
            "wo": round_f32r(perm(wo[sl, :])),
            "bq": round_f32r(bq[sl]),
            "bk": round_f32r(bk[sl]),
        })
    return in_maps


def combine(results, wv_full, bv, wo_full, bo):
    y = np.zeros((B, S, E), np.float32)
    for c in range(NCORES):
        y[c // GH] += results[c]["yT"].T
    y += bv @ wo_full + bo
    return y


def kernel(inputs, wq, bq, wk, bk, wv, bv, wo, bo, _run_kwargs=None):
    inputs = np.asarray(inputs, np.float32)
    wq, bq = np.asarray(wq, np.float32), np.asarray(bq, np.float32)
    wk, bk = np.asarray(wk, np.float32), np.asarray(bk, np.float32)
    wv, bv = np.asarray(wv, np.float32), np.asarray(bv, np.float32)
    wo, bo = np.asarray(wo, np.float32), np.asarray(bo, np.float32)

    nc = get_nc()
    in_maps = make_in_maps(inputs, wq, bq, wk, bk, wv, wo)
    res = run_bass_kernel_spmd(nc, in_maps, list(range(NCORES)),
                               **(_run_kwargs or {}))
    y = combine(res.results, wv, bv, wo, bo)
    if _run_kwargs:
        kernel.last_result = res
    return y



# revision 5
# speedup vs baseline: 1.0542x; 1.0542x over previous
"""Multi-head self-attention (B=2, S=2048, E=1024, H=16, D=64) on 8 NeuronCores.

Sharding: core c -> (batch b = c // 4, head group g = c % 4).  Each core
computes Q/K/V projections for its 4 heads (column-parallel), attention, and
a partial output projection (row-parallel); the host sums the 4 partials per
batch.  All device activations live in "transposed space" (feature on the
partition dim) so every matmul contracts along partitions:

  Q^T = Wq_g^T @ X^T + bq    [256, 2048]  (bias via per-partition tensor_scalar)
  K^T = Wk_g^T @ X^T         [256, 2048]  (bk provably cancels in softmax)
  V   = X @ Wv_g             [2048, 256]  (ones column appended per head)
  S^T = K_h @ Q_h^T / 8      [2048, 2048] per head, kc-tile-wise
  P^T = exp(S^T)             (no max-subtraction: scores ~N(0,1))
  O'^T = [V_h | 1]^T @ P^T   [65, q]  (row 64 = softmax denominators)
  O^T  = O'[0:64] / O'[64]
  Y^T  = Wo_g^T @ O^T        [1024, 2048] partial, host-summed per batch

Schedule notes (from HW microbenchmarks):
  - K-dim switches on the PE (K=64 scores vs K=128 rest) cost ~110ns each, so
    scores pairs are batched 2 kc at a time (PSUM caps the group at 2) and all
    other matmuls are emitted in homogeneous K=128 runs.
  - The two heads of a pair run as concurrent 64-row PE tiles (~222ns/pair).
  - ScalarE exp is the co-bottleneck (~1114ns per [128,1024] tile); the
    emission order keeps one exp tile ready per ~1.1us continuously.
  - PSUM: 2 scores tiles (4 banks) + 4 acc/projection ring slots (4 banks).
"""

from contextlib import ExitStack

import numpy as np

import concourse.bass as bass
import concourse.tile as tile
from concourse import bacc, mybir
from concourse.bass_utils import run_bass_kernel_spmd

B, S, E, H, D = 2, 2048, 1024, 16, 64
NCORES = 8
GH = 4            # heads per core
DC = GH * D       # head-dim columns per core (256)
EC = E // 128     # 8 e-chunks
KC = S // 128     # 16 k-chunks
F32 = mybir.dt.float32
MM_DT = mybir.dt.float16
EXP_FUNC = mybir.ActivationFunctionType.Exp
SCALE = 1.0 / np.sqrt(np.float32(D))


def round_f32r(a):
    return np.ascontiguousarray(a, np.float32).astype(np.float16)


def _emit(nc, tc, ctx, xT, wq, wk, wv, wo, bq, yT):
    sb_big = ctx.enter_context(tc.tile_pool(name="sb_big", bufs=1))
    sb_p = ctx.enter_context(tc.tile_pool(name="sb_p", bufs=24))
    sb_norm = ctx.enter_context(tc.tile_pool(name="sb_norm", bufs=4))
    sb_y = ctx.enter_context(tc.tile_pool(name="sb_y", bufs=3))
    ps_sco = ctx.enter_context(tc.tile_pool(name="ps_sco", bufs=2, space="PSUM"))
    ps_acc = ctx.enter_context(tc.tile_pool(name="ps_acc", bufs=2, space="PSUM"))

    xT_t = sb_big.tile([128, EC, S], MM_DT)
    wq_t = sb_big.tile([128, EC, DC], MM_DT)
    wk_t = sb_big.tile([128, EC, DC], MM_DT)
    wv_t = sb_big.tile([128, EC, DC], MM_DT)
    wo_t = sb_big.tile([128, 2, E], MM_DT)
    bq_t = sb_big.tile([128, 2], F32)
    qT_t = sb_big.tile([128, 2, S], MM_DT)
    kT_t = sb_big.tile([128, 2, S], MM_DT)
    v_t = sb_big.tile([128, KC, GH, D + 1], MM_DT)
    o_t = sb_big.tile([128, 2, S], MM_DT)

    # Preload the exp table set while input DMAs run (first real exp would
    # otherwise pay the ~2.7us ACT_TABLE_LOAD in the critical path).
    warm = sb_norm.tile([1, 2], F32, tag="warm")
    nc.vector.memset(warm[:, :], 0.0)
    nc.scalar.activation(out=warm[:, :], in_=warm[:, :], func=EXP_FUNC)

    # Input DMAs.  wk first (kT chains gate the exp stream), then xT in
    # 512-column chunks so the first qk chains start after ~1MB, not 4MB.
    # Issue on sync/gpsimd/vector queues only (scalar must stay free).
    nc.sync.dma_start(out=wk_t[:, :, :],
                      in_=wk.rearrange("p (c d) -> p c d", c=EC))
    nc.gpsimd.dma_start(out=wq_t[:, :, :],
                        in_=wq.rearrange("p (c d) -> p c d", c=EC))
    nc.scalar.dma_start(out=bq_t[:, :], in_=bq)
    qs = [nc.sync, nc.gpsimd]
    for sc in range(4):
        for ec in range(EC):
            # first chunks also use the scalar queue (idle until first exp)
            q = nc.scalar if sc == 0 and ec >= 6 else qs[ec % 2]
            q.dma_start(
                out=xT_t[:, ec, sc * 512:(sc + 1) * 512],
                in_=xT[:, ec * S + sc * 512: ec * S + (sc + 1) * 512])
    nc.sync.dma_start(out=wv_t[:, :, :],
                      in_=wv.rearrange("p (c d) -> p c d", c=EC))
    nc.gpsimd.dma_start(out=wo_t[:, :, :],
                        in_=wo.rearrange("p (c e) -> p c e", c=2))
    for kc in range(KC):
        nc.vector.memset(v_t[:, kc, :, D:D + 1], 1.0)

    # ---- emitters --------------------------------------------------------
    qk_state = {}

    def qk_half(proj, dc, sc, half):
        # psum[d, s] += W[e, d].T @ X^T[e, s]; Q bias applied in the cast.
        w_t, dst = ((wq_t, qT_t), (wk_t, kT_t))[proj]
        key = (proj, dc, sc)
        if half == 0:
            qk_state[key] = ps_acc.tile([128, 512], F32, tag="acc", bufs=4,
                                        name="ps_qk")
        ps = qk_state[key]
        for ec in (range(4) if half == 0 else range(4, EC)):
            nc.tensor.matmul(ps[:, :],
                             lhsT=w_t[:, ec, dc * 128:(dc + 1) * 128],
                             rhs=xT_t[:, ec, sc * 512:(sc + 1) * 512],
                             start=(ec == 0), stop=(ec == EC - 1))
        if half == 1:
            out = dst[:, dc, sc * 512:(sc + 1) * 512]
            if proj == 0:
                nc.vector.tensor_scalar_add(out=out, in0=ps[:, :],
                                            scalar1=bq_t[:, dc:dc + 1])
            else:
                nc.vector.tensor_copy(out=out, in_=ps[:, :])
            del qk_state[key]

    def qk_chain(proj, dc, sc):
        qk_half(proj, dc, sc, 0)
        qk_half(proj, dc, sc, 1)

    v_state = {}

    def v_half(kc, half):
        # psum[s, d] += X^T[e, s].T @ Wv[e, d]
        if half == 0:
            v_state[kc] = ps_acc.tile([128, 512], F32, tag="acc", bufs=4,
                                      name="ps_v")
        ps = v_state[kc]
        for ec in (range(4) if half == 0 else range(4, EC)):
            nc.tensor.matmul(ps[:, 0:DC],
                             lhsT=xT_t[:, ec, kc * 128:(kc + 1) * 128],
                             rhs=wv_t[:, ec, :],
                             start=(ec == 0), stop=(ec == EC - 1))
        if half == 1:
            nc.vector.tensor_copy(
                out=v_t[:, kc, :, 0:D],
                in_=ps[:, 0:DC].rearrange("p (h d) -> p h d", h=GH))
            del v_state[kc]

    def v_chain(kc):
        v_half(kc, 0)
        v_half(kc, 1)

    def pair(qc, hc, kc):
        # Head pair scores: two concurrent 64-row PE tiles, one ACTIVATE.
        sco = ps_sco.tile([128, 2, 512], F32, name="sco")
        for hp in range(2):
            po = hp * 64
            nc.tensor.matmul(
                sco[:, hp, :],
                lhsT=kT_t[po:po + 64, hc, kc * 128:(kc + 1) * 128],
                rhs=qT_t[po:po + 64, hc, qc * 512:(qc + 1) * 512],
                start=True, stop=True)
        pT = sb_p.tile([128, 2, 512], MM_DT)
        nc.scalar.activation(out=pT[:, :, :], in_=sco[:, :, :], func=EXP_FUNC,
                             scale=float(SCALE))
        return pT

    def pv_alloc():
        return [ps_acc.tile([128, 512], F32, tag="acc", bufs=4, name=f"acc{j}")
                for j in range(2)]

    def pv_kc(accs, hc, pTs, kc):
        for hp in range(2):
            h = 2 * hc + hp
            nc.tensor.matmul(
                accs[hp][0:D + 1, :],
                lhsT=v_t[:, kc, h, :],
                rhs=pTs[kc][:, hp, :],
                start=(kc == 0), stop=(kc == KC - 1))

    def attention_norm(qc, hc, accs):
        for hp in range(2):
            po = hp * 64
            rs = sb_norm.tile([1, 512], F32, tag="rs")
            nc.vector.tensor_copy(out=rs[:, :], in_=accs[hp][D:D + 1, :])
            inv_r = sb_norm.tile([1, 512], F32, tag="inv")
            nc.vector.reciprocal_approx_fast(out=inv_r[:, :], in_=rs[:, :])
            brd = sb_norm.tile([64, 512], F32, tag="brd")
            nc.gpsimd.partition_broadcast(brd[:, :], inv_r[:, :])
            nc.vector.tensor_mul(
                o_t[po:po + 64, hc, qc * 512:(qc + 1) * 512],
                accs[hp][0:D, :],
                brd[:, :])

    def y_group(qc, ec, copy_eng=None):
        # psum[e, s] += Wo[c, e].T @ O^T[c, s]
        yp = ps_acc.tile([128, 512], F32, tag="acc", bufs=4, name="yp")
        for cc in range(2):
            nc.tensor.matmul(
                yp[:, :],
                lhsT=wo_t[:, cc, ec * 128:(ec + 1) * 128],
                rhs=o_t[:, cc, qc * 512:(qc + 1) * 512],
                start=(cc == 0), stop=(cc == 1))
        ys = sb_y.tile([128, 512], MM_DT)
        if copy_eng == "scalar":
            nc.scalar.copy(out=ys[:, :], in_=yp[:, :])
        else:
            nc.vector.tensor_copy(out=ys[:, :], in_=yp[:, :])
        (nc.sync if ec % 2 == 0 else nc.gpsimd).dma_start(
            out=yT[ec * 128:(ec + 1) * 128, qc * 512:(qc + 1) * 512],
            in_=ys[:, :])

    # ---- schedule --------------------------------------------------------
    # blocks in (qc, hc) order; block bi's scores overlap block bi-1's PV.
    #
    # Ring discipline (ps_acc, 4 slots): each block allocates its 2 PV accs
    # at g0, front-loads all 16 PV matmuls into groups g0-g3, and emits the
    # norm right after g3 -- so the accs release mid-block and the chain
    # fillers (qk/v/y, each a self-contained alloc->release run) never ring-
    # wait on work that depends on them.  V chains all live in block 0 (+2
    # at the very start of block 1), since PV(block0) consumes V in block 1.
    blocks = [(0, 0), (1, 0), (2, 0), (3, 0), (0, 1), (1, 1), (2, 1), (3, 1)]

    def F(fn, *a):
        return lambda: fn(*a)

    # per-block chain fillers: {bi: {group: [closure, ...]}}.
    # Deadlines: qT[dc=hc][qc] before block (qc,hc) starts; kT[1][sc] before
    # block 4 reaches kc=4*sc; y(qc) after norm(qc,1) (emitted at g4 of the
    # following block).  pre-norm groups (g0/g1) may carry at most the two
    # chain allocs that immediately follow pv_alloc (V14/V15 in block 1).
    fillers = {
        1: {0: [F(v_chain, 14)], 1: [F(v_chain, 15)],
            4: [F(qk_chain, 0, 0, 2)], 6: [F(qk_chain, 1, 1, 0)]},
        2: {4: [F(qk_chain, 0, 0, 3)], 6: [F(qk_chain, 1, 1, 1)]},
        3: {4: [F(qk_chain, 0, 1, 0)], 6: [F(qk_chain, 1, 1, 2)]},
        4: {4: [F(qk_chain, 1, 1, 3)], 6: [F(qk_chain, 0, 1, 1)]},
        5: {4: [F(qk_chain, 0, 1, 2)], 6: [F(y_group, 0, 0)],
            7: [F(y_group, 0, 1)]},
        6: {4: [F(qk_chain, 0, 1, 3)], 5: [F(y_group, 0, 2), F(y_group, 0, 3)],
            6: [F(y_group, 0, 4), F(y_group, 0, 5)],
            7: [F(y_group, 0, 6), F(y_group, 0, 7)]},
        7: {4: [F(y_group, 1, 0), F(y_group, 1, 1)],
            5: [F(y_group, 1, 2), F(y_group, 1, 3)],
            6: [F(y_group, 1, 4), F(y_group, 1, 5)],
            7: [F(y_group, 1, 6), F(y_group, 1, 7)]},
    }

    # block 0 (startup): no PV yet; stagger kT/qT/V chains between pairs.
    # kT[0][sc] gates pairs kc=4sc..4sc+3; all V except 14/15 must land here.
    qk_chain(1, 0, 0)           # kT dc0 sc0
    qk_chain(0, 0, 0)           # qT dc0 qc0
    b0_fill = {
        0: [F(qk_chain, 1, 0, 1)],
        1: [F(v_chain, 0)],
        2: [F(qk_chain, 1, 0, 2), F(v_chain, 1)],
        3: [F(v_chain, 2), F(v_chain, 3)],
        4: [F(qk_chain, 1, 0, 3), F(v_chain, 4)],
        5: [F(qk_chain, 0, 0, 1), F(v_chain, 5), F(v_chain, 6)],
        6: [F(v_chain, 7), F(v_chain, 8), F(v_chain, 9)],
        7: [F(v_chain, 10), F(v_chain, 11), F(v_chain, 12), F(v_chain, 13)],
    }
    pts_prev = []
    for g in range(8):
        pts_prev += [pair(0, 0, 2 * g), pair(0, 0, 2 * g + 1)]
        for fn in b0_fill.get(g, []):
            fn()

    prev_block = (0, 0)
    for bi in range(1, len(blocks)):
        qc, hc = blocks[bi]
        pqc, phc = prev_block
        accs = pv_alloc()
        pts_cur = []
        fl = fillers.get(bi, {})
        for g in range(8):
            pts_cur += [pair(qc, hc, 2 * g), pair(qc, hc, 2 * g + 1)]
            if g < 4:
                for kc in range(4 * g, 4 * g + 4):
                    pv_kc(accs, phc, pts_prev, kc)
            for fn in fl.get(g, []):
                fn()
            if g == 3:
                attention_norm(pqc, phc, accs)
        pts_prev = pts_cur
        prev_block = (qc, hc)

    # final: PV of the last block paces with its exp stream; y(2) fills,
    # then norm, then y(3) with copies split across scalar/vector.
    accs = pv_alloc()
    for g in range(8):
        pv_kc(accs, prev_block[1], pts_prev, 2 * g)
        pv_kc(accs, prev_block[1], pts_prev, 2 * g + 1)
        y_group(2, g)
    attention_norm(prev_block[0], prev_block[1], accs)
    for ec in range(EC):
        y_group(3, ec, copy_eng="scalar" if ec % 2 else None)


_cached_nc = None


def _build():
    nc = bacc.Bacc(trn_type="TRN2", target_bir_lowering=False)
    xT = nc.dram_tensor("xT", [128, EC * S], MM_DT, kind="ExternalInput").ap()
    wq = nc.dram_tensor("wq", [128, EC * DC], MM_DT, kind="ExternalInput").ap()
    wk = nc.dram_tensor("wk", [128, EC * DC], MM_DT, kind="ExternalInput").ap()
    wv = nc.dram_tensor("wv", [128, EC * DC], MM_DT, kind="ExternalInput").ap()
    wo = nc.dram_tensor("wo", [128, 2 * E], MM_DT, kind="ExternalInput").ap()
    bq = nc.dram_tensor("bq", [128, 2], F32, kind="ExternalInput").ap()
    yT = nc.dram_tensor("yT", [E, S], MM_DT, kind="ExternalOutput").ap()
    with tile.TileContext(nc) as tc:
        with ExitStack() as ctx:
            _emit(nc, tc, ctx, xT, wq, wk, wv, wo, bq, yT)
    nc.compile()
    return nc


def get_nc():
    global _cached_nc
    if _cached_nc is None:
        _cached_nc = _build()
    return _cached_nc


def make_in_maps(inputs, wq, bq, wk, wv, wo):
    in_maps = []
    for c in range(NCORES):
        b, g = divmod(c, GH)
        sl = slice(g * DC, (g + 1) * DC)

        def perm(a):
            # [C*128, N] -> [128, C*N] with SBUF chunk-major free dim
            cN = a.shape[0] // 128
            return np.ascontiguousarray(
                a.reshape(cN, 128, a.shape[1]).transpose(1, 0, 2).reshape(
                    128, cN * a.shape[1]))

        in_maps.append({
            "xT": round_f32r(perm(np.ascontiguousarray(inputs[b].T))),
            "wq": round_f32r(perm(wq[:, sl])),
            "wk": round_f32r(perm(wk[:, sl])),
            "wv": round_f32r(perm(wv[:, sl])),
            "wo": round_f32r(perm(wo[sl, :])),
            "bq": np.ascontiguousarray(
                bq[sl].reshape(2, 128).T, np.float32),
        })
    return in_maps


def combine(results, bv, wo_full, bo):
    y = np.zeros((B, S, E), np.float32)
    for c in range(NCORES):
        y[c // GH] += results[c]["yT"].astype(np.float32).T
    y += bv.astype(np.float32) @ wo_full + bo
    return y


def kernel(inputs, wq, bq, wk, bk, wv, bv, wo, bo, _run_kwargs=None):
    # bk provably cancels: softmax over keys is invariant to the per-query
    # term (q+bq)@bk, and k enters the computation only through the scores.
    inputs = np.asarray(inputs, np.float32)
    wq, bq = np.asarray(wq, np.float32), np.asarray(bq, np.float32)
    wk = np.asarray(wk, np.float32)
    wv, bv = np.asarray(wv, np.float32), np.asarray(bv, np.float32)
    wo, bo = np.asarray(wo, np.float32), np.asarray(bo, np.float32)

    nc = get_nc()
    in_maps = make_in_maps(inputs, wq, bq, wk, wv, wo)
    res = run_bass_kernel_spmd(nc, in_maps, list(range(NCORES)),
                               **(_run_kwargs or {}))
    y = combine(res.results, bv, wo, bo)
    if _run_kwargs:
        kernel.last_result = res
    return y


# revision 11
# speedup vs baseline: 1.0750x; 1.0197x over previous
"""Multi-head self-attention (B=2, S=2048, E=1024, H=16, D=64) on 8 NeuronCores.

Sharding: core c -> (batch b = c // 4, head group g = c % 4).  Each core
computes Q/K/V projections for its 4 heads (column-parallel), attention, and
a partial output projection (row-parallel); the host sums the 4 partials per
batch.  All device activations live in "transposed space" (feature on the
partition dim) so every matmul contracts along partitions:

  Q^T = Wq_g^T @ X^T + bq    [256, 2048]  (bias via per-partition tensor_scalar)
  K^T = Wk_g^T @ X^T         [256, 2048]  (bk provably cancels in softmax)
  V   = X @ Wv_g             [2048, 256]  (ones column appended per head)
  S^T = K_h @ Q_h^T / 8      [2048, 2048] per head, kc-tile-wise
  P^T = exp(S^T)             (no max-subtraction: scores ~N(0,1))
  O'^T = [V_h | 1]^T @ P^T   [65, q]  (row 64 = softmax denominators)
  O^T  = O'[0:64] / O'[64]
  Y^T  = Wo_g^T @ O^T        [1024, 2048] partial, host-summed per batch

Schedule notes (from HW microbenchmarks):
  - K-dim switches on the PE (K=64 scores vs K=128 rest) cost ~110ns each, so
    scores pairs are batched 2 kc at a time (PSUM caps the group at 2) and all
    other matmuls are emitted in homogeneous K=128 runs.
  - The two heads of a pair run as concurrent 64-row PE tiles (~222ns/pair).
  - ScalarE exp is the co-bottleneck (~1114ns per [128,1024] tile); the
    emission order keeps one exp tile ready per ~1.1us continuously.
  - PSUM: 2 scores tiles (4 banks) + 4 acc/projection ring slots (4 banks).
"""

from contextlib import ExitStack

import numpy as np

import concourse.bass as bass
import concourse.tile as tile
from concourse import bacc, mybir
from concourse.bass_utils import run_bass_kernel_spmd

B, S, E, H, D = 2, 2048, 1024, 16, 64
NCORES = 8
GH = 4            # heads per core
DC = GH * D       # head-dim columns per core (256)
EC = E // 128     # 8 e-chunks
KC = S // 128     # 16 k-chunks
F32 = mybir.dt.float32
MM_DT = mybir.dt.float16
EXP_FUNC = mybir.ActivationFunctionType.Exp
SCALE = 1.0 / np.sqrt(np.float32(D))


def round_f32r(a):
    return np.ascontiguousarray(a, np.float32).astype(np.float16)


def _emit(nc, tc, ctx, xT, wq, wk, wv, wo, bq, yT):
    sb_big = ctx.enter_context(tc.tile_pool(name="sb_big", bufs=1))
    sb_p = ctx.enter_context(tc.tile_pool(name="sb_p", bufs=24))
    sb_norm = ctx.enter_context(tc.tile_pool(name="sb_norm", bufs=4))
    sb_y = ctx.enter_context(tc.tile_pool(name="sb_y", bufs=3))
    ps_sco = ctx.enter_context(tc.tile_pool(name="ps_sco", bufs=2, space="PSUM"))
    ps_acc = ctx.enter_context(tc.tile_pool(name="ps_acc", bufs=2, space="PSUM"))

    xT_t = sb_big.tile([128, EC, S], MM_DT)
    wq_t = sb_big.tile([128, EC, DC], MM_DT)
    wk_t = sb_big.tile([128, EC, DC], MM_DT)
    wv_t = sb_big.tile([128, EC, DC], MM_DT)
    wo_t = sb_big.tile([128, 2, E], MM_DT)
    bq_t = sb_big.tile([128, 2], F32)
    qT_t = sb_big.tile([128, 2, S], MM_DT)
    kT_t = sb_big.tile([128, 2, S], MM_DT)
    v_t = sb_big.tile([128, KC, GH, D + 1], MM_DT)
    o_t = sb_big.tile([128, 2, S], MM_DT)

    # Preload the exp table set while input DMAs run (first real exp would
    # otherwise pay the ~2.7us ACT_TABLE_LOAD in the critical path).
    warm = sb_norm.tile([1, 2], F32, tag="warm")
    nc.vector.memset(warm[:, :], 0.0)
    nc.scalar.activation(out=warm[:, :], in_=warm[:, :], func=EXP_FUNC)

    # Input DMAs.  wk first (kT chains gate the exp stream), then xT in
    # 512-column chunks so the first qk chains start after ~1MB, not 4MB.
    # Issue on sync/gpsimd/vector queues only (scalar must stay free).
    nc.sync.dma_start(out=wk_t[:, :, :],
                      in_=wk.rearrange("p (c d) -> p c d", c=EC))
    nc.gpsimd.dma_start(out=wq_t[:, :, :],
                        in_=wq.rearrange("p (c d) -> p c d", c=EC))
    nc.scalar.dma_start(out=bq_t[:, :], in_=bq)
    qs = [nc.sync, nc.gpsimd]
    for sc in range(4):
        for ec in range(EC):
            qs[ec % 2].dma_start(
                out=xT_t[:, ec, sc * 512:(sc + 1) * 512],
                in_=xT[:, ec * S + sc * 512: ec * S + (sc + 1) * 512])
    nc.sync.dma_start(out=wv_t[:, :, :],
                      in_=wv.rearrange("p (c d) -> p c d", c=EC))
    nc.gpsimd.dma_start(out=wo_t[:, :, :],
                        in_=wo.rearrange("p (c e) -> p c e", c=2))
    for kc in range(KC):
        nc.vector.memset(v_t[:, kc, :, D:D + 1], 1.0)

    # ---- emitters --------------------------------------------------------
    qk_state = {}

    def qk_half(proj, dc, sc, half):
        # psum[d, s] += W[e, d].T @ X^T[e, s]; Q bias applied in the cast.
        w_t, dst = ((wq_t, qT_t), (wk_t, kT_t))[proj]
        key = (proj, dc, sc)
        if half == 0:
            qk_state[key] = ps_acc.tile([128, 512], F32, tag="acc", bufs=4,
                                        name="ps_qk")
        ps = qk_state[key]
        for ec in (range(4) if half == 0 else range(4, EC)):
            nc.tensor.matmul(ps[:, :],
                             lhsT=w_t[:, ec, dc * 128:(dc + 1) * 128],
                             rhs=xT_t[:, ec, sc * 512:(sc + 1) * 512],
                             start=(ec == 0), stop=(ec == EC - 1))
        if half == 1:
            out = dst[:, dc, sc * 512:(sc + 1) * 512]
            if proj == 0:
                nc.vector.tensor_scalar_add(out=out, in0=ps[:, :],
                                            scalar1=bq_t[:, dc:dc + 1])
            else:
                nc.vector.tensor_copy(out=out, in_=ps[:, :])
            del qk_state[key]

    def qk_chain(proj, dc, sc):
        qk_half(proj, dc, sc, 0)
        qk_half(proj, dc, sc, 1)

    v_state = {}

    def v_half(kc, half):
        # psum[s, d] += X^T[e, s].T @ Wv[e, d]
        if half == 0:
            v_state[kc] = ps_acc.tile([128, 512], F32, tag="acc", bufs=4,
                                      name="ps_v")
        ps = v_state[kc]
        for ec in (range(4) if half == 0 else range(4, EC)):
            nc.tensor.matmul(ps[:, 0:DC],
                             lhsT=xT_t[:, ec, kc * 128:(kc + 1) * 128],
                             rhs=wv_t[:, ec, :],
                             start=(ec == 0), stop=(ec == EC - 1))
        if half == 1:
            nc.vector.tensor_copy(
                out=v_t[:, kc, :, 0:D],
                in_=ps[:, 0:DC].rearrange("p (h d) -> p h d", h=GH))
            del v_state[kc]

    def v_chain(kc):
        v_half(kc, 0)
        v_half(kc, 1)

    def pair(qc, hc, kc):
        # Head pair scores: two concurrent 64-row PE tiles, one ACTIVATE.
        sco = ps_sco.tile([128, 2, 512], F32, name="sco")
        for hp in range(2):
            po = hp * 64
            nc.tensor.matmul(
                sco[:, hp, :],
                lhsT=kT_t[po:po + 64, hc, kc * 128:(kc + 1) * 128],
                rhs=qT_t[po:po + 64, hc, qc * 512:(qc + 1) * 512],
                start=True, stop=True)
        pT = sb_p.tile([128, 2, 512], MM_DT)
        nc.scalar.activation(out=pT[:, :, :], in_=sco[:, :, :], func=EXP_FUNC,
                             scale=float(SCALE))
        return pT

    def pv_alloc():
        return [ps_acc.tile([128, 512], F32, tag="acc", bufs=4, name=f"acc{j}")
                for j in range(2)]

    def pv_kc(accs, hc, pTs, kc):
        for hp in range(2):
            h = 2 * hc + hp
            nc.tensor.matmul(
                accs[hp][0:D + 1, :],
                lhsT=v_t[:, kc, h, :],
                rhs=pTs[kc][:, hp, :],
                start=(kc == 0), stop=(kc == KC - 1))

    def attention_norm(qc, hc, accs):
        for hp in range(2):
            po = hp * 64
            rs = sb_norm.tile([1, 512], F32, tag="rs")
            nc.vector.tensor_copy(out=rs[:, :], in_=accs[hp][D:D + 1, :])
            inv_r = sb_norm.tile([1, 512], F32, tag="inv")
            nc.vector.reciprocal_approx_fast(out=inv_r[:, :], in_=rs[:, :])
            brd = sb_norm.tile([64, 512], F32, tag="brd")
            nc.gpsimd.partition_broadcast(brd[:, :], inv_r[:, :])
            nc.vector.tensor_mul(
                o_t[po:po + 64, hc, qc * 512:(qc + 1) * 512],
                accs[hp][0:D, :],
                brd[:, :])

    def y_group(qc, ec, copy_eng=None):
        # psum[e, s] += Wo[c, e].T @ O^T[c, s]
        yp = ps_acc.tile([128, 512], F32, tag="acc", bufs=4, name="yp")
        for cc in range(2):
            nc.tensor.matmul(
                yp[:, :],
                lhsT=wo_t[:, cc, ec * 128:(ec + 1) * 128],
                rhs=o_t[:, cc, qc * 512:(qc + 1) * 512],
                start=(cc == 0), stop=(cc == 1))
        ys = sb_y.tile([128, 512], MM_DT)
        if copy_eng == "scalar":
            nc.scalar.copy(out=ys[:, :], in_=yp[:, :])
        else:
            nc.vector.tensor_copy(out=ys[:, :], in_=yp[:, :])
        (nc.sync if ec % 2 == 0 else nc.gpsimd).dma_start(
            out=yT[ec * 128:(ec + 1) * 128, qc * 512:(qc + 1) * 512],
            in_=ys[:, :])

    # ---- schedule --------------------------------------------------------
    # blocks in (qc, hc) order; block bi's scores overlap block bi-1's PV.
    #
    # Ring discipline (ps_acc, 4 slots): each block allocates its 2 PV accs
    # at g0, front-loads all 16 PV matmuls into groups g0-g3, and emits the
    # norm right after g3 -- so the accs release mid-block and the chain
    # fillers (qk/v/y, each a self-contained alloc->release run) never ring-
    # wait on work that depends on them.  V chains all live in block 0 (+2
    # at the very start of block 1), since PV(block0) consumes V in block 1.
    blocks = [(0, 0), (1, 0), (2, 0), (3, 0), (0, 1), (1, 1), (2, 1), (3, 1)]

    def F(fn, *a):
        return lambda: fn(*a)

    # per-block chain fillers: {bi: {group: [closure, ...]}}.
    # Deadlines: qT[dc=hc][qc] before block (qc,hc) starts; kT[1][sc] before
    # block 4 reaches kc=4*sc; y(qc) after norm(qc,1) (emitted at g4 of the
    # following block).  pre-norm groups (g0/g1) may carry at most the two
    # chain allocs that immediately follow pv_alloc (V14/V15 in block 1).
    fillers = {
        1: {0: [F(v_chain, 14)], 1: [F(v_chain, 15)],
            4: [F(qk_chain, 0, 0, 2)], 6: [F(qk_chain, 1, 1, 0)]},
        2: {4: [F(qk_chain, 0, 0, 3)], 6: [F(qk_chain, 1, 1, 1)]},
        3: {4: [F(qk_chain, 0, 1, 0)], 6: [F(qk_chain, 1, 1, 2)]},
        4: {4: [F(qk_chain, 1, 1, 3)], 6: [F(qk_chain, 0, 1, 1)]},
        5: {4: [F(qk_chain, 0, 1, 2), F(y_group, 0, 0)],
            5: [F(y_group, 0, 1), F(y_group, 0, 2)],
            6: [F(y_group, 0, 3), F(y_group, 0, 4)],
            7: [F(y_group, 0, 5), F(y_group, 0, 6)]},
        6: {4: [F(y_group, 0, 7), F(qk_chain, 0, 1, 3)],
            5: [F(y_group, 1, 0), F(y_group, 1, 1)],
            6: [F(y_group, 1, 2), F(y_group, 1, 3)],
            7: [F(y_group, 1, 4), F(y_group, 1, 5)]},
        7: {4: [F(y_group, 1, 6), F(y_group, 1, 7)],
            5: [F(y_group, 2, 0), F(y_group, 2, 1)],
            6: [F(y_group, 2, 2), F(y_group, 2, 3)],
            7: [F(y_group, 2, 4), F(y_group, 2, 5)]},
    }

    # block 0 (startup): no PV yet; stagger kT/qT/V chains between pairs.
    # kT[0][sc] gates pairs kc=4sc..4sc+3; all V except 14/15 must land here.
    qk_chain(1, 0, 0)           # kT dc0 sc0
    qk_chain(0, 0, 0)           # qT dc0 qc0
    b0_fill = {
        0: [F(qk_chain, 1, 0, 1)],
        1: [F(v_chain, 0)],
        2: [F(qk_chain, 1, 0, 2), F(v_chain, 1)],
        3: [F(v_chain, 2), F(v_chain, 3)],
        4: [F(qk_chain, 1, 0, 3), F(v_chain, 4)],
        5: [F(qk_chain, 0, 0, 1), F(v_chain, 5), F(v_chain, 6)],
        6: [F(v_chain, 7), F(v_chain, 8), F(v_chain, 9)],
        7: [F(v_chain, 10), F(v_chain, 11), F(v_chain, 12), F(v_chain, 13)],
    }
    pts_prev = []
    for g in range(8):
        pts_prev += [pair(0, 0, 2 * g), pair(0, 0, 2 * g + 1)]
        for fn in b0_fill.get(g, []):
            fn()

    prev_block = (0, 0)
    for bi in range(1, len(blocks)):
        qc, hc = blocks[bi]
        pqc, phc = prev_block
        accs = pv_alloc()
        pts_cur = []
        fl = fillers.get(bi, {})
        for g in range(8):
            pts_cur += [pair(qc, hc, 2 * g), pair(qc, hc, 2 * g + 1)]
            if g < 4:
                for kc in range(4 * g, 4 * g + 4):
                    pv_kc(accs, phc, pts_prev, kc)
            for fn in fl.get(g, []):
                fn()
            if g == 3:
                attention_norm(pqc, phc, accs)
        pts_prev = pts_cur
        prev_block = (qc, hc)

    # final: PV of the last block paces with its exp stream; remaining y(2)
    # fills, then norm, then y(3) with copies split across scalar/vector.
    accs = pv_alloc()
    for g in range(8):
        pv_kc(accs, prev_block[1], pts_prev, 2 * g)
        pv_kc(accs, prev_block[1], pts_prev, 2 * g + 1)
        if g >= 6:
            y_group(2, g)
    attention_norm(prev_block[0], prev_block[1], accs)
    for ec in range(EC):
        y_group(3, ec, copy_eng="scalar" if ec % 2 else None)


_cached_nc = None


def _build():
    nc = bacc.Bacc(trn_type="TRN2", target_bir_lowering=False)
    xT = nc.dram_tensor("xT", [128, EC * S], MM_DT, kind="ExternalInput").ap()
    wq = nc.dram_tensor("wq", [128, EC * DC], MM_DT, kind="ExternalInput").ap()
    wk = nc.dram_tensor("wk", [128, EC * DC], MM_DT, kind="ExternalInput").ap()
    wv = nc.dram_tensor("wv", [128, EC * DC], MM_DT, kind="ExternalInput").ap()
    wo = nc.dram_tensor("wo", [128, 2 * E], MM_DT, kind="ExternalInput").ap()
    bq = nc.dram_tensor("bq", [128, 2], F32, kind="ExternalInput").ap()
    yT = nc.dram_tensor("yT", [E, S], MM_DT, kind="ExternalOutput").ap()
    with tile.TileContext(nc) as tc:
        with ExitStack() as ctx:
            _emit(nc, tc, ctx, xT, wq, wk, wv, wo, bq, yT)
    nc.compile()
    return nc


def get_nc():
    global _cached_nc
    if _cached_nc is None:
        _cached_nc = _build()
    return _cached_nc


def make_in_maps(inputs, wq, bq, wk, wv, wo):
    in_maps = []
    for c in range(NCORES):
        b, g = divmod(c, GH)
        sl = slice(g * DC, (g + 1) * DC)

        def perm(a):
            # [C*128, N] -> [128, C*N] with SBUF chunk-major free dim
            cN = a.shape[0] // 128
            return np.ascontiguousarray(
                a.reshape(cN, 128, a.shape[1]).transpose(1, 0, 2).reshape(
                    128, cN * a.shape[1]))

        in_maps.append({
            "xT": round_f32r(perm(np.ascontiguousarray(inputs[b].T))),
            "wq": round_f32r(perm(wq[:, sl])),
            "wk": round_f32r(perm(wk[:, sl])),
            "wv": round_f32r(perm(wv[:, sl])),
            "wo": round_f32r(perm(wo[sl, :])),
            "bq": np.ascontiguousarray(
                bq[sl].reshape(2, 128).T, np.float32),
        })
    return in_maps


def combine(results, bv, wo_full, bo):
    y = np.zeros((B, S, E), np.float32)
    for c in range(NCORES):
        y[c // GH] += results[c]["yT"].astype(np.float32).T
    y += bv.astype(np.float32) @ wo_full + bo
    return y


def kernel(inputs, wq, bq, wk, bk, wv, bv, wo, bo, _run_kwargs=None):
    # bk provably cancels: softmax over keys is invariant to the per-query
    # term (q+bq)@bk, and k enters the computation only through the scores.
    inputs = np.asarray(inputs, np.float32)
    wq, bq = np.asarray(wq, np.float32), np.asarray(bq, np.float32)
    wk = np.asarray(wk, np.float32)
    wv, bv = np.asarray(wv, np.float32), np.asarray(bv, np.float32)
    wo, bo = np.asarray(wo, np.float32), np.asarray(bo, np.float32)

    nc = get_nc()
    in_maps = make_in_maps(inputs, wq, bq, wk, wv, wo)
    res = run_bass_kernel_spmd(nc, in_maps, list(range(NCORES)),
                               **(_run_kwargs or {}))
    y = combine(res.results, bv, wo, bo)
    if _run_kwargs:
        kernel.last_result = res
    return y
